# revision 6
# baseline (speedup 1.0000x reference)
"""NodePairGaussianKernel Trainium2 kernel.

Computes, per (batch, bin): A = x*msk (512 nodes x 16 feat);
D2[m,n] = |A_m|^2 + |A_n|^2 - 2 A_m.A_n; out = exp(-0.1*sqrt(max(D2, 1e-6))).

Sharding: batch dim (8) -> 8 NeuronCores, fully data parallel.

Per-core plan (bins = 64, grouped into 8 supers of 8 bins):
 - host passes xT (64,16,512) fp32 and msk (64,512) fp32 per core.
 - dense prep per super: X8 (128,512) = 8 bins x 16 f-rows; mask-mult;
   S8 = A*A; S split hi/lo bf16; na row-sums via indicator matmul (K=128)
   into psum (8,512); na split hi/lo bf16, DMA-gathered into na_flat (2,32768).
 - G matmuls: fp32r strips (4 bins x 32-aligned 16-row strips per (128,512)
   tile); per bin, per m-block: K=16 fp32r mm (-2*A^T x A^T) + K=2 na[n] mm +
   K=2 na[m] mm accumulate D2 into a psum bank.
 - evict psum->SBUF stage with fused max(.,eps) (DVE tensor_scalar),
   then pin the diagonal stripe to eps (affine_select) to kill fp32r
   cancellation noise.
 - batched ACT passes per super: sqrt (in-place), exp(-0.1*) (in-place),
   table sets stay resident across each whole pass.
 - per-bin 1MB output DMAs.
"""

import sys

sys.path.insert(0, "/opt/trn_rl_repo")

from contextlib import ExitStack

import numpy as np
import ml_dtypes

import concourse.bass as bass
import concourse.tile as tile
from concourse import mybir
from concourse.bass_utils import run_bass_kernel_spmd

B, BINS, NNODE, F = 8, 64, 512, 16
P = 128
EPS = 1e-6
DIST_MULT = 0.1
SUPERS = 8          # supers per core
BINS_PER_SUPER = 8
DIAG_FIX = False

f32 = mybir.dt.float32
f32r = mybir.dt.float32r
bf16 = mybir.dt.bfloat16


def _split_multi_waits(nc):
    """The 64B instruction format has one sync-wait slot; walrus rejects
    instructions carrying several. Split extras into single-wait NoOps."""
    ctr = 0
    for f in nc.m.functions:
        for b in f.blocks:
            insts = b.instructions
            i = 0
            while i < len(insts):
                inst = insts[i]
                si = inst.sync_info
                if si is not None and si.on_wait and len(si.on_wait) > 1:
                    waits = list(si.on_wait)
                    for w in waits[:-1]:
                        nop = mybir.InstNoOp(
                            name=f"I-wsplit-{ctr}",
                            engine=inst.engine,
                            ins=[],
                            outs=[],
                            sync_info=mybir.SyncInfo(on_wait=[w], on_update=[]),
                        )
                        ctr += 1
                        insts.insert(i, nop)
                        i += 1
                    inst.sync_info = mybir.SyncInfo(
                        on_wait=[waits[-1]], on_update=list(si.on_update)
                    )
                i += 1
    return ctr


def build_nc():
    nc = bass.Bass(trn_type="TRN2")

    xt = nc.declare_dram_parameter("xt", [BINS, F, NNODE], f32, isOutput=False)
    mskp = nc.declare_dram_parameter("msk", [BINS, NNODE], f32, isOutput=False)
    ind8 = nc.declare_dram_parameter("ind8", [P, BINS_PER_SUPER], bf16, isOutput=False)
    ones2 = nc.declare_dram_parameter("ones2", [2, 128 + NNODE], bf16, isOutput=False)
    outp = nc.declare_dram_parameter("out", [BINS, NNODE, NNODE], f32, isOutput=True)

    xt_flat = xt.rearrange("k f n -> (k f) n")

    with tile.TileContext(nc) as tc, ExitStack() as ctx:
        const_pool = ctx.enter_context(tc.tile_pool(name="consts", bufs=1))
        prep = ctx.enter_context(tc.tile_pool(name="prep", bufs=2))
        strips = ctx.enter_context(tc.tile_pool(name="strips", bufs=3))
        napool = ctx.enter_context(tc.tile_pool(name="napool", bufs=2))
        stage_pool = ctx.enter_context(tc.tile_pool(name="stage", bufs=3))
        psD_pool = ctx.enter_context(tc.tile_pool(name="psD", bufs=3, space="PSUM"))
        psNA_pool = ctx.enter_context(tc.tile_pool(name="psNA", bufs=1, space="PSUM"))

        ind_sb = const_pool.tile([P, BINS_PER_SUPER], bf16)
        nc.sync.dma_start(out=ind_sb[:], in_=ind8[:])
        ones_sb = const_pool.tile([2, 128 + NNODE], bf16)
        nc.sync.dma_start(out=ones_sb[:], in_=ones2[:])

        for s in range(SUPERS):
            bin0 = s * BINS_PER_SUPER
            # ---- dense prep: 8 bins x 16 f-rows
            X8 = prep.tile([P, NNODE], f32, tag="X8")
            nc.sync.dma_start(out=X8[:], in_=xt_flat[s * P : (s + 1) * P, :])
            mrep = prep.tile([P, NNODE], f32, tag="mrep")
            for j in range(BINS_PER_SUPER):
                nc.sync.dma_start(
                    out=mrep[16 * j : 16 * j + 16, :],
                    in_=mskp[bin0 + j : bin0 + j + 1, :].to_broadcast((16, NNODE)),
                )
            nc.vector.tensor_tensor(
                out=X8[:], in0=X8[:], in1=mrep[:], op=mybir.AluOpType.mult
            )
            S8 = prep.tile([P, NNODE], f32, tag="S8")
            nc.vector.tensor_tensor(
                out=S8[:], in0=X8[:], in1=X8[:], op=mybir.AluOpType.mult
            )
            S_hi = prep.tile([P, NNODE], bf16, tag="S_hi")
            nc.vector.tensor_copy(out=S_hi[:], in_=S8[:])
            S_lo = prep.tile([P, NNODE], bf16, tag="S_lo")
            nc.vector.tensor_tensor(
                out=S_lo[:], in0=S8[:], in1=S_hi[:], op=mybir.AluOpType.subtract
            )
            # ---- na via indicator matmul: psum_na[j, n] = sum_f S[16j+f, n]
            ps_na = psNA_pool.tile([BINS_PER_SUPER, NNODE], f32, tag="psna")
            nc.tensor.matmul(ps_na[:], ind_sb[:], S_hi[:], start=True, stop=False)
            nc.tensor.matmul(ps_na[:], ind_sb[:], S_lo[:], start=False, stop=True)
            na_sb = napool.tile([BINS_PER_SUPER, NNODE], f32, tag="na_sb")
            nc.vector.tensor_copy(out=na_sb[:], in_=ps_na[:])
            na_hi = napool.tile([BINS_PER_SUPER, NNODE], bf16, tag="na_hi")
            nc.vector.tensor_copy(out=na_hi[:], in_=na_sb[:])
            na_lo = napool.tile([BINS_PER_SUPER, NNODE], bf16, tag="na_lo")
            nc.vector.tensor_tensor(
                out=na_lo[:], in0=na_sb[:], in1=na_hi[:], op=mybir.AluOpType.subtract
            )
            # gather na rows into the per-super flat layout (partition remap via DMA)
            na_flat = napool.tile([2, BINS_PER_SUPER * NNODE], bf16, tag="na_flat")
            for j in range(BINS_PER_SUPER):
                sl = slice(j * NNODE, (j + 1) * NNODE)
                nc.sync.dma_start(out=na_flat[0:1, sl], in_=na_hi[j : j + 1, :])
                nc.sync.dma_start(out=na_flat[1:2, sl], in_=na_lo[j : j + 1, :])

            # ---- fp32r strip tiles: 2 groups x 4 bins at 32-aligned strips
            stages = []
            for g in range(2):
                stage = stage_pool.tile([P, 4 * 4 * NNODE], f32, tag="stage")
                stages.append(stage)
                Ast = strips.tile([P, NNODE], f32, tag="Ast")
                for j in range(4):
                    nc.sync.dma_start(
                        out=Ast[32 * j : 32 * j + 16, :],
                        in_=X8[16 * (4 * g + j) : 16 * (4 * g + j) + 16, :],
                    )
                Ar = strips.tile([P, NNODE], f32r, tag="Ar")
                nc.vector.tensor_copy(out=Ar[:], in_=Ast[:])
                An = strips.tile([P, NNODE], f32r, tag="An")
                nc.vector.tensor_scalar_mul(An[:], Ast[:], -2.0)

                for j in range(4):
                    lbin = 4 * g + j          # bin within super
                    ksl = slice(32 * j, 32 * j + 16)
                    for half in range(2):
                        psD = psD_pool.tile([P, 2 * NNODE], f32, tag="psD")
                        for ih in range(2):
                            i = 2 * half + ih
                            bank = psD[:, NNODE * ih : NNODE * (ih + 1)]
                            nc.tensor.matmul(
                                bank,
                                An[ksl, P * i : P * (i + 1)],
                                Ar[ksl, :],
                                start=True,
                                stop=False,
                                tile_position=(32 * j, 0),
                            )
                            nc.tensor.matmul(
                                bank,
                                ones_sb[:, 0:128],
                                na_flat[:, lbin * NNODE : (lbin + 1) * NNODE],
                                start=False,
                                stop=False,
                            )
                            nc.tensor.matmul(
                                bank,
                                na_flat[:, lbin * NNODE + P * i : lbin * NNODE + P * (i + 1)],
                                ones_sb[:, 128:],
                                start=False,
                                stop=True,
                            )
                        # evict with fused max(., eps)
                        ssl = slice(j * 4 * NNODE + half * 2 * NNODE,
                                    j * 4 * NNODE + (half + 1) * 2 * NNODE)
                        nc.vector.tensor_scalar_max(stage[:, ssl], psD[:], EPS)

            # ---- batched activations, grouped by table set across both halves
            for st in stages:
                nc.scalar.activation(
                    out=st[:], in_=st[:], func=mybir.ActivationFunctionType.Sqrt
                )
            for st in stages:
                nc.scalar.activation(
                    out=st[:], in_=st[:],
                    func=mybir.ActivationFunctionType.Exp, scale=-DIST_MULT,
                )
            # ---- output DMAs: one per bin (1MB contiguous in DRAM)
            for g in range(2):
                for j in range(4):
                    gbin = bin0 + 4 * g + j
                    src_ap = stages[g][:, j * 4 * NNODE : (j + 1) * 4 * NNODE].rearrange(
                        "p (i n) -> p i n", i=4
                    )
                    dst = outp[gbin].rearrange("(i p) n -> p i n", p=P)
                    nc.sync.dma_start(out=dst, in_=src_ap)

    _split_multi_waits(nc)
    return nc


_NC_CACHE = None


def kernel(x_msg_binned: np.ndarray, msk: np.ndarray) -> np.ndarray:
    global _NC_CACHE
    if _NC_CACHE is None:
        _NC_CACHE = build_nc()
    nc = _NC_CACHE

    ind8_np = np.zeros((P, BINS_PER_SUPER), dtype=ml_dtypes.bfloat16)
    for j in range(BINS_PER_SUPER):
        ind8_np[16 * j : 16 * j + 16, j] = 1.0
    ones2_np = np.ones((2, 128 + NNODE), dtype=ml_dtypes.bfloat16)

    in_maps = []
    for c in range(B):
        xt_c = np.ascontiguousarray(
            x_msg_binned[c].transpose(0, 2, 1).astype(np.float32)
        )
        msk_c = np.ascontiguousarray(msk[c, :, :, 0].astype(np.float32))
        in_maps.append(
            {"xt": xt_c, "msk": msk_c, "ind8": ind8_np, "ones2": ones2_np}
        )

    res = run_bass_kernel_spmd(nc, in_maps, list(range(B)))
    out = np.stack([r["out"] for r in res.results], axis=0)
    return out[..., None].astype(np.float32)


if __name__ == "__main__":
    rng = np.random.default_rng(0)
    x = rng.standard_normal((B, BINS, NNODE, F)).astype(np.float32)
    m = np.ones((B, BINS, NNODE, 1), dtype=np.float32)
    o = kernel(x_msg_binned=x, msk=m)
    print(o.shape, o.dtype)
